# revision 44
# baseline (speedup 1.0000x reference)
"""Multi-head causal self-attention on 8 Trainium2 NeuronCores.

Sharding: core c -> (batch b = c//2, head-group hg = c%2): data-parallel over
the 4 batches x tensor-parallel over 2 groups of 8 heads. c_attn is
column-parallel, fc_out row-parallel (Megatron); the row-parallel partial sums
are reduced on the host during the gather/unshard step (fp16 partials,
fp32 accumulation).

All-16-bit compute with host-side pre-transpose of x:
 - host ships x^T/weights as fp16 (no on-chip transposes or rounding copies)
 - Q/K fp16; exp output + V in bf16 (range covers exp(q.q/8) diagonal tails)
 - softmax denominators fused into the PV matmul via a ones-column on V
 - "superstream" attention: the two heads of a pair share each eps PSUM tile
   (partition halves 0:64 / 64:128 = disjoint PE row groups), so their K=64
   energy matmuls run concurrently in the array (~57 ns/MM measured vs ~228
   serialized); eps pools ping-pong per k-tile to keep ScalarE's exp stream
   fed
 - softmax normalize: one tensor_copy frees the oT bank immediately,
   reciprocal_approx_fast (base-partition-0 slice; exact reciprocal is a
   ~3 us single-lane microcoded op) off the critical path, PE ones-broadcast
   and the final scale deferred several rounds
 - phase-pipelined schedule: QKV projection of s-block st+1 and the deferred
   fc_out matmuls are metered into the ACT-bound attention stretches of
   q-window qt, so the (in-order) PE queue never stalls on exp.
"""
import numpy as np
from collections import deque
from contextlib import ExitStack

import concourse.bass as bass
import concourse.mybir as mybir
import concourse.tile as tile
from concourse import bacc
from concourse.bass_utils import run_bass_kernel_spmd

dt = mybir.dt
AF = mybir.ActivationFunctionType

B, S, E, H = 4, 2048, 1024, 16
D = 64            # head dim
HL = 8            # heads per core
DL = HL * D       # 512, local attention width
ECH = E // 128    # 8 contraction chunks over embed dim
NQT = S // 512    # 4 q-tiles of 512
NST = S // 128    # 16 s-subtiles of 128
SCALE = 1.0 / np.sqrt(np.float32(D))
EGRP = 2          # energy k-tiles per exp() group (2 PSUM banks)

_CACHE = {}


def _build(reps=1, loop=1, upto=3, act_slim=False, skip_affsel=False,
           skip_norm=False, skip_energy=False, skip_pv=False,
           skip_exp=False, norm_mode="full"):
    nc = bacc.Bacc("TRN2")
    f16, bf16, f32, f32r = dt.float16, dt.bfloat16, dt.float32, dt.float32r

    xT = nc.dram_tensor("xT", [E, S], f16, kind="ExternalInput")
    wq = nc.dram_tensor("wq", [E, DL], f16, kind="ExternalInput")
    wk = nc.dram_tensor("wk", [E, DL], f16, kind="ExternalInput")
    wv = nc.dram_tensor("wv", [E, DL], f16, kind="ExternalInput")
    wo = nc.dram_tensor("wo", [DL, E], f16, kind="ExternalInput")
    bqk = nc.dram_tensor("bqk", [2 * DL], f32, kind="ExternalInput")
    bv = nc.dram_tensor("bv", [DL], f32, kind="ExternalInput")
    bo = nc.dram_tensor("bo", [E], f32, kind="ExternalInput")
    out = nc.dram_tensor("out", [S, E], f16, kind="ExternalOutput")

    def bcast_dram(row_ap, parts):
        return bass.AP(tensor=row_ap.tensor, offset=row_ap.offset,
                       ap=[[0, parts]] + list(row_ap.ap[1:]))

    with tile.TileContext(nc) as tc, ExitStack() as top:
        top.enter_context(nc.allow_low_precision(
            reason="16-bit attention compute is intentional"))
        persist = top.enter_context(tc.tile_pool(name="persist", bufs=1))

        # QT/KT: [d, s] pair-packed fp16: pair p=(head 2p, 2p+1) -> partitions
        # (0:64, 64:128), free block p*2048 + s
        QT = persist.tile([128, 4 * S], f16)
        KT = persist.tile([128, 4 * S], f16)
        AT = persist.tile([128, 4 * S], f16)
        # V: [s, d] bf16 per (head l, s-subtile t): free (l*16+t)*65,
        # cols 0:64 = V, col 64 = 1.0 (fused softmax denominator)
        V = persist.tile([128, HL * NST * 65], bf16)
        # consts: [0:128) ones, [128:136) bqk, [136:648) bv bcast
        consts = persist.tile([128, 648], f32)
        ones_f = consts[:, 0:128]
        nc.vector.memset(ones_f, 1.0)
        bqk_sb = consts[:, 128:136]
        nc.sync.dma_start(out=bqk_sb, in_=bqk.rearrange("(c p) -> p c", p=128))
        bv_bc = consts[:, 136:648]
        nc.sync.dma_start(out=bv_bc, in_=bcast_dram(bv[None, :], 128))
        ones_r = persist.tile([128, 64], f32r)
        nc.vector.tensor_copy(out=ones_r[:], in_=ones_f[:, 0:64])

        def _rep_body():
            ctx = ExitStack()
            pw = ctx.enter_context(tc.tile_pool(name="pw", bufs=1))
            p_pt = ctx.enter_context(tc.tile_pool(name="p_pt", bufs=3))
            p_s = ctx.enter_context(tc.tile_pool(name="p_s", bufs=2))
            p_o = ctx.enter_context(tc.tile_pool(name="p_o", bufs=2))
            ps_e0 = ctx.enter_context(
                tc.tile_pool(name="ps_e0", bufs=1, space="PSUM"))
            ps_e1 = ctx.enter_context(
                tc.tile_pool(name="ps_e1", bufs=1, space="PSUM"))
            ps_o = ctx.enter_context(
                tc.tile_pool(name="ps_o", bufs=2, space="PSUM"))
            ps_x = ctx.enter_context(
                tc.tile_pool(name="ps_x", bufs=2, space="PSUM"))

            xT_sb = pw.tile([128, ECH, S], f16)
            wq_sb = pw.tile([128, ECH, DL], f16)
            wk_sb = pw.tile([128, ECH, DL], f16)
            wv_sb = pw.tile([128, ECH, DL], f16)
            wo_sb = pw.tile([128, 4, E], f16)
            bo_bc = pw.tile([128, E], f32)

            xTr = xT.rearrange("(eo p) s -> p eo s", p=128)
            nc.sync.dma_start(out=wq_sb[:],
                              in_=wq.rearrange("(eo p) d -> p eo d", p=128))
            nc.sync.dma_start(out=xT_sb[:, :, 0:512], in_=xTr[:, :, 0:512])
            nc.sync.dma_start(out=wk_sb[:],
                              in_=wk.rearrange("(eo p) d -> p eo d", p=128))
            nc.sync.dma_start(out=wv_sb[:],
                              in_=wv.rearrange("(eo p) d -> p eo d", p=128))
            for st in range(1, NQT):
                nc.sync.dma_start(out=xT_sb[:, :, st * 512:(st + 1) * 512],
                                  in_=xTr[:, :, st * 512:(st + 1) * 512])
            nc.sync.dma_start(out=wo_sb[:],
                              in_=wo.rearrange("(co p) n -> p co n", p=128))
            nc.sync.dma_start(out=bo_bc[:], in_=bcast_dram(bo[None, :], 128))

            Vv = V[:].rearrange("p (l t c) -> p l t c", l=HL, c=65)
            eps_pools = [ps_e0, ps_e1]
            eps_fresh = [2, 2]  # first-use garbage memsets per pool
            # normalize tails (bc matmul + AT mul) deferred a few rounds so
            # the PE never waits on the DVE reciprocal: [rounds_left, closure]
            deferred = deque()

            def proj_stream(st):
                """QKV projections for s-window st. Yields after each MM.
                Emission order: pair-0's Q+K chunks, then V, then the later
                pairs' chunks — so qt=st attention can start ASAP after."""

                def qk_chunk(dch):
                    w_sb = wq_sb if dch < 4 else wk_sb
                    dsl = slice((dch % 4) * 128, (dch % 4) * 128 + 128)
                    pq = ps_x.tile([128, 512], f32, tag="px", name="pq")
                    for ech in range(ECH):
                        nc.tensor.matmul(
                            pq[:], w_sb[:, ech, dsl],
                            xT_sb[:, ech, st * 512:(st + 1) * 512],
                            start=(ech == 0), stop=(ech == ECH - 1))
                        yield
                    dest = QT if dch < 4 else KT
                    pair = dch % 4
                    nc.vector.tensor_scalar_add(
                        out=dest[:, pair * S + st * 512:
                                 pair * S + (st + 1) * 512],
                        in0=pq[:], scalar1=bqk_sb[:, dch:dch + 1])

                def v_chunk(sub):
                    t = st * 4 + sub
                    pv = ps_x.tile([128, 512], f32, tag="px", name="pv")
                    for ech in range(ECH):
                        nc.tensor.matmul(
                            pv[:], xT_sb[:, ech, t * 128:(t + 1) * 128],
                            wv_sb[:, ech, :],
                            start=(ech == 0), stop=(ech == ECH - 1))
                        yield
                    nc.vector.tensor_add(
                        out=Vv[:, :, t, 0:64],
                        in0=pv[:].rearrange("p (l d) -> p l d", d=64),
                        in1=bv_bc.rearrange("p (l d) -> p l d", d=64))
                    nc.vector.tensor_copy(out=Vv[:, :, t, 64],
                                          in_=ones_f[:, 0:HL])

                for g in [qk_chunk(d) for d in range(8)] + \
                         [v_chunk(s) for s in range(4)]:
                    yield from g
                yield

            def pair_stream(lp, qt):
                """Attention for head pair lp (heads 2lp, 2lp+1) over one
                q-window. The two heads' energy matmuls share one eps tile
                (rows 0:64 / 64:128 -> disjoint PE row-groups, so they run
                concurrently in the array); eps pools ping-pong per k-tile.
                Yields per PE-matmul slot."""
                pair = lp
                n_kt = 4 * (qt + 1)
                oTs = [ps_o.tile([65, 512], f32, tag="oT", name=f"oT{h}")
                       for h in range(2)]
                q0 = pair * S + qt * 512
                for kt in range(n_kt):
                    pool = eps_pools[kt % 2]
                    eps = pool.tile([128, 2, 512], f32, tag=f"eps{kt % 2}")
                    if eps_fresh[kt % 2] > 0:
                        nc.vector.memset(eps[:], 0.0)
                        eps_fresh[kt % 2] -= 1
                    a = (kt - 4 * qt) * 128 if kt >= 4 * qt else 0
                    if not skip_energy:
                        for h in range(2):
                            pb = h * 64
                            nc.tensor.matmul(
                                eps[:, h, a:],
                                KT[pb:pb + 64, pair * S + kt * 128:
                                   pair * S + (kt + 1) * 128],
                                QT[pb:pb + 64, q0 + a:q0 + 512],
                                start=True, stop=True)
                    yield
                    ao = 448 if act_slim else a
                    pt = p_pt.tile([128, 2, 512], bf16, tag="pt")
                    if not skip_exp:
                        nc.scalar.activation(out=pt[:, :, ao:],
                                             in_=eps[:, :, ao:],
                                             func=AF.Exp, scale=float(SCALE))
                    for h in range(2):
                        l = 2 * lp + h
                        if kt >= 4 * qt and not skip_affsel:
                            # keep where q_local - k_local >= 0
                            nc.gpsimd.affine_select(
                                out=pt[:, h, a:], in_=pt[:, h, a:],
                                compare_op=mybir.AluOpType.is_ge,
                                fill=0.0, base=0,
                                pattern=[[1, 512 - a]],
                                channel_multiplier=-1)
                        if not skip_pv:
                            nc.tensor.matmul(
                                oTs[h][:, a:], V[:, (l * NST + kt) * 65:
                                                 (l * NST + kt) * 65 + 65],
                                pt[:, h, a:],
                                start=(kt == 0), stop=(kt == n_kt - 1))
                        yield
                # softmax normalization: row 64 of oT = denominators.
                if skip_norm:
                    yield
                    return
                for h in range(2):
                    pb = h * 64
                    oT = oTs[h]
                    # one copy frees the oT bank; recip runs on the SBUF copy
                    osb = p_s.tile([128, 512], f32, tag="osb")
                    nc.vector.tensor_copy(out=osb[0:65, :], in_=oT[:])
                    # approx recip needs base_partition 0 (silently no-ops
                    # from 64); only row 64 of the output is used
                    rd = p_s.tile([128, 512], f32, tag="rd")
                    nc.vector.reciprocal_approx_fast(out=rd[0:65, :],
                                                     in_=osb[0:65, :])

                    def norm_tail(rd=rd, osb=osb, pb=pb, q0=q0):
                        if norm_mode == "noat":
                            return
                        if norm_mode == "nobc":
                            nc.vector.tensor_copy(
                                out=AT[pb:pb + 64, q0:q0 + 512],
                                in_=osb[0:64, :])
                            return
                        bc = ps_x.tile([128, 512], f32, tag="px")
                        nc.tensor.matmul(bc[0:64, :], ones_f[64:65, 0:64],
                                         rd[64:65, :], start=True, stop=True)
                        nc.vector.tensor_mul(out=AT[pb:pb + 64, q0:q0 + 512],
                                             in0=bc[0:64, :], in1=osb[0:64, :])

                    deferred.append([6, norm_tail])
                    yield

            def pump_deferred(force=False):
                for d in deferred:
                    d[0] -= 1
                while deferred and (force or deferred[0][0] <= 0):
                    deferred.popleft()[1]()

            def fc_stream(qt):
                """fc_out for q-window qt (row-parallel partial) + out DMA."""
                for st_loc in range(4):
                    st = qt * 4 + st_loc
                    o_sb = p_o.tile([128, E], f16, tag="o_sb")
                    for half in range(2):
                        pf = ps_x.tile([128, 512], f32, tag="px")
                        for dch in range(4):
                            nc.tensor.matmul(
                                pf[:],
                                AT[:, dch * S + st * 128:
                                   dch * S + (st + 1) * 128],
                                wo_sb[:, dch, half * 512:(half + 1) * 512],
                                start=(dch == 0), stop=(dch == 3))
                            yield
                        nc.vector.tensor_add(
                            out=o_sb[:, half * 512:(half + 1) * 512],
                            in0=pf[:],
                            in1=bo_bc[:, half * 512:(half + 1) * 512])
                    nc.sync.dma_start(out=out[st * 128:(st + 1) * 128, :],
                                      in_=o_sb[:])
                    yield

            PROJ_STEPS = 8 * ECH + 4 * ECH + 1   # 97
            FC_STEPS = 4 * (2 * 4 + 1)           # 36

            # lead-in: projections for s-window 0
            for _ in proj_stream(0):
                pass

            if upto < 2:
                for st in range(1, NQT):
                    for _ in proj_stream(st):
                        pass
                ctx.close()
                return

            fillers = deque()
            for qt in range(NQT):
                budget = 0
                if qt + 1 < NQT:
                    fillers.append(proj_stream(qt + 1))
                    budget += PROJ_STEPS
                if qt == NQT - 1 and upto >= 3:
                    for q2 in range(NQT - 1):
                        fillers.append(fc_stream(q2))
                        budget += FC_STEPS
                n_kt = 4 * (qt + 1)
                total_rounds = 4 * (3 * n_kt + 2)
                r = 0
                done_f = 0
                for lp in range(4):
                    for _ in pair_stream(lp, qt):
                        r += 1
                        pump_deferred()
                        target = min(budget, budget * (r + 4) // total_rounds)
                        while done_f < target and fillers:
                            if next(fillers[0], "end") == "end":
                                fillers.popleft()
                            else:
                                done_f += 1
                # drain fillers at segment end (proj st+1 must complete
                # before qt+1's energy matmuls enter the PE queue)
                while fillers:
                    if next(fillers[0], "end") == "end":
                        fillers.popleft()
                pump_deferred(force=True)
            if upto >= 3:
                for _ in fc_stream(NQT - 1):
                    pass
            ctx.close()

        if loop > 1:
            with tc.For_i(0, loop, 1):
                _rep_body()
        else:
            for _rep in range(reps):
                _rep_body()

    nc.finalize()
    return nc


def _in_maps(x, w_attn, b_attn, w_out, b_out):
    x = np.asarray(x, np.float32)
    w_attn = np.asarray(w_attn, np.float32)
    b_attn = np.asarray(b_attn, np.float32)
    w_out = np.asarray(w_out, np.float32)
    b_out = np.asarray(b_out, np.float32)
    zeros_e = np.zeros((E,), np.float32)
    maps = []
    for c in range(8):
        b, hg = c // 2, c % 2
        sq = slice(0 * E + hg * DL, 0 * E + (hg + 1) * DL)
        sk = slice(1 * E + hg * DL, 1 * E + (hg + 1) * DL)
        sv = slice(2 * E + hg * DL, 2 * E + (hg + 1) * DL)
        maps.append({
            "xT": np.ascontiguousarray(x[b].T.astype(np.float16)),
            "wq": np.ascontiguousarray(w_attn[:, sq].astype(np.float16)),
            "wk": np.ascontiguousarray(w_attn[:, sk].astype(np.float16)),
            "wv": np.ascontiguousarray(w_attn[:, sv].astype(np.float16)),
            "wo": np.ascontiguousarray(
                w_out[hg * DL:(hg + 1) * DL, :].astype(np.float16)),
            "bqk": np.concatenate([b_attn[sq], b_attn[sk]]),
            "bv": np.ascontiguousarray(b_attn[sv]),
            "bo": b_out if hg == 0 else zeros_e,
        })
    return maps


def _run(x, w_attn, b_attn, w_out, b_out, trace=False):
    if "nc" not in _CACHE:
        _CACHE["nc"] = _build()
    maps = _in_maps(x, w_attn, b_attn, w_out, b_out)
    res = run_bass_kernel_spmd(_CACHE["nc"], maps, list(range(8)), trace=trace)
    outs = np.empty((B, S, E), np.float32)
    for b in range(B):
        outs[b] = (res.results[2 * b]["out"].astype(np.float32)
                   + res.results[2 * b + 1]["out"].astype(np.float32))
    return outs, res


def kernel(x, w_attn, b_attn, w_out, b_out):
    outs, _ = _run(x, w_attn, b_attn, w_out, b_out, trace=False)
    return outs
